# revision 1
# baseline (speedup 1.0000x reference)
"""MultiHeadKANAttention Trainium2 kernel (8 NeuronCores, SPMD).

Strategy:
  - Token-sharded KANLinear QKV: each core computes silu(x) and the 8
    unnormalized cubic B-spline basis planes for its 256-token slab
    (hat-function recursion split across ACT/DVE/GPSIMD), then a fused
    K=9216 bf16 matmul against streamed weights produces qkv[256, 3072].
    The out-feature dim is processed in two groups: [ke ko v] first, then
    [qe qo], so the k/v all-to-all + unpack + transposes overlap the q-group
    matmul.
  - RoPE applied on the QKV PSUM banks (weights row-permuted so even/odd
    rotation pairs form contiguous 512-column blocks; 1/sqrt(d) folded into
    the k-rows; B-spline 1/6 normalization and spline_scaler folded into the
    spline weights on the host).
  - Two AllToAll waves (1MB + 0.5MB) re-shard to 2 heads per core with all
    2048 tokens.
  - Attention per head with transposed-scores layout: scoresT[sk,sq] = k^T q,
    exp on ACT (no max subtraction needed in fp32: |scores| <= ~25), AV
    matmul with an appended ones-row computing the softmax denominator for
    free, normalization applied at the ctx stage via gpsimd partition
    broadcast of the reciprocal.
  - Partial output projection per core (its 128 ctx channels x full out_w
    column slice); host sums the 8 partials and adds bias.

All matmuls bf16 (1 cyc/row). Verified vs the jax reference: rel_l2 ~ 0.9e-2.
"""
import os
import numpy as np
import ml_dtypes

S = 2048
F = 1024
H = 16
HD = 64
O = 3 * F
CORES = 8
SLAB = S // CORES   # 256
NCH = 72            # 8 silu + 64 spline chunks
OKV = 2048          # [ke ko v] columns, group 0
OQ = 1024           # [qe qo] columns, group 1
BF16 = ml_dtypes.bfloat16

_PI = None


def _build_pi():
    """pi[new_row] = original qkv out_feature row. New order:
    [ke_all(512) | ko_all(512) | qe_all(512) | qo_all(512) | v_all(1024)]."""
    pi = np.zeros(O, dtype=np.int64)
    for h in range(H):
        base = h * 192
        for i in range(32):
            pi[0 * 512 + h * 32 + i] = base + 64 + 2 * i
            pi[1 * 512 + h * 32 + i] = base + 64 + 2 * i + 1
            pi[2 * 512 + h * 32 + i] = base + 2 * i
            pi[3 * 512 + h * 32 + i] = base + 2 * i + 1
        for j in range(HD):
            pi[2048 + h * 64 + j] = base + 128 + j
    return pi


def _host_prep(x, base_weight, spline_weight, spline_scaler, out_w, rot_cos, rot_sin):
    global _PI
    if _PI is None:
        _PI = _build_pi()
    pi = _PI
    x2 = np.asarray(x, np.float32).reshape(S, F)
    xT = np.ascontiguousarray(x2.T)

    W = np.asarray(base_weight, np.float32)[pi].copy()
    sw = np.asarray(spline_weight, np.float32) * np.asarray(spline_scaler, np.float32)[:, :, None]
    sw = sw[pi] * np.float32(1.0 / 6.0)
    W[0:1024] *= np.float32(0.125)      # fold 1/sqrt(HD) into k rows
    sw[0:1024] *= np.float32(0.125)

    # group 0 = pi-rows [0:2048] (ke ko qe qo), group 1 = [2048:3072] (v)
    wm0 = np.empty((NCH, 128, OKV), np.float32)
    wm1 = np.empty((NCH, 128, OQ), np.float32)
    for t in range(NCH):
        if t < 8:
            blk = W[:, t * 128:(t + 1) * 128]
        else:
            fb, c = (t - 8) // 8, (t - 8) % 8
            blk = sw[:, fb * 128:(fb + 1) * 128, c]
        wm0[t] = blk[0:2048].T
        wm1[t] = blk[2048:3072].T
    wm0 = wm0.astype(BF16)
    wm1 = wm1.astype(BF16)

    Ct = np.tile(np.asarray(rot_cos, np.float32), (1, H))
    St = np.tile(np.asarray(rot_sin, np.float32), (1, H))
    wo_all = np.asarray(out_w, np.float32)

    in_maps = []
    for c in range(CORES):
        sl = slice(c * SLAB, (c + 1) * SLAB)
        in_maps.append({
            "xs": np.ascontiguousarray(xT[:, sl]),
            "cs": np.ascontiguousarray(Ct[sl]),
            "sn": np.ascontiguousarray(St[sl]),
            "wm0": wm0,
            "wm1": wm1,
            "wo": np.ascontiguousarray(wo_all[:, c * 128:(c + 1) * 128].T).astype(BF16),
        })
    return in_maps


def _build_program(single_core=False, reps=1):
    from contextlib import ExitStack
    import concourse.bass as bass
    import concourse.mybir as mybir
    import concourse.tile as tile
    from concourse import bacc
    from concourse.masks import make_identity

    dt = mybir.dt
    op = mybir.AluOpType
    AF = mybir.ActivationFunctionType
    PSUM = bass.MemorySpace.PSUM

    nc = bacc.Bacc("TRN2", target_bir_lowering=False, debug=False,
                   enable_asserts=False, num_devices=CORES)

    xs_d = nc.declare_dram_parameter("xs", [F, SLAB], dt.float32, isOutput=False)
    cs_d = nc.declare_dram_parameter("cs", [SLAB, 512], dt.float32, isOutput=False)
    sn_d = nc.declare_dram_parameter("sn", [SLAB, 512], dt.float32, isOutput=False)
    wm0_d = nc.declare_dram_parameter("wm0", [NCH, 128, OKV], dt.bfloat16, isOutput=False)
    wm1_d = nc.declare_dram_parameter("wm1", [NCH, 128, OQ], dt.bfloat16, isOutput=False)
    wo_d = nc.declare_dram_parameter("wo", [128, F], dt.bfloat16, isOutput=False)
    out_d = nc.declare_dram_parameter("out", [S, F], dt.float32, isOutput=True)
    if reps > 1:
        nc.declare_dram_parameter("reptag", [1, reps], dt.float32, isOutput=False)

    with tile.TileContext(nc, num_cores=CORES) as tc, ExitStack() as ctx:
        const = ctx.enter_context(tc.tile_pool(name="const", bufs=1))
        acts = ctx.enter_context(tc.tile_pool(name="acts", bufs=1))
        tmp = ctx.enter_context(tc.tile_pool(name="tmp", bufs=2))
        wstream = ctx.enter_context(tc.tile_pool(name="wstream", bufs=6))
        ropes = ctx.enter_context(tc.tile_pool(name="ropes", bufs=1))
        attn = ctx.enter_context(tc.tile_pool(name="attn", bufs=1))
        attnbuf = ctx.enter_context(tc.tile_pool(name="attnbuf", bufs=2))

        # ---- constants ----
        ident = const.tile([128, 128], dt.bfloat16, tag="ident", name="ident")
        make_identity(nc, ident[:])
        cs_sb = [const.tile([128, 512], dt.float32, tag=f"cs{tt}", name=f"cs{tt}") for tt in range(2)]
        sn_sb = [const.tile([128, 512], dt.float32, tag=f"sn{tt}", name=f"sn{tt}") for tt in range(2)]
        wo_sb = const.tile([128, F], dt.bfloat16, tag="wo", name="wo")
        xf_sb = [const.tile([128, SLAB], dt.float32, tag=f"xf{fb}", name=f"xf{fb}") for fb in range(8)]
        for fb in range(8):
            nc.sync.dma_start(xf_sb[fb][:], xs_d[fb * 128:(fb + 1) * 128, :])
        habias = {}
        for i in range(1, 11):
            bt = const.tile([128, 1], dt.float32, tag=f"bias{i}", name=f"bias{i}")
            nc.vector.memset(bt[:], float(5.5 - i))
            habias[i] = bt

        for rep in range(reps):
            a2a_kq_i = nc.dram_tensor(f"a2a_kq_i{rep}", [CORES, SLAB, 256], dt.bfloat16)
            a2a_kq_o = nc.dram_tensor(f"a2a_kq_o{rep}", [CORES, SLAB, 256], dt.bfloat16)
            a2a_v_i = nc.dram_tensor(f"a2a_v_i{rep}", [CORES, SLAB, 128], dt.bfloat16)
            a2a_v_o = nc.dram_tensor(f"a2a_v_o{rep}", [CORES, SLAB, 128], dt.bfloat16)

            # ---- phase 1: silu + b-spline basis chunks ----
            silu_sb = [acts.tile([128, SLAB], dt.bfloat16, tag=f"silu{fb}", name=f"silu{fb}")
                       for fb in range(8)]
            for fb in range(8):
                sg = tmp.tile([128, SLAB], dt.float32, tag="sg", name="sg")
                nc.scalar.activation(sg[:], xf_sb[fb][:], AF.Sigmoid)
                nc.vector.tensor_tensor(silu_sb[fb][:], xf_sb[fb][:], sg[:], op.mult)

            bs_sb = [[acts.tile([128, SLAB], dt.bfloat16, tag=f"bs{fb}_{c}", name=f"bs{fb}_{c}")
                      for c in range(8)] for fb in range(8)]
            for fb in range(8):
                xf = xf_sb[fb]
                u = tmp.tile([128, SLAB], dt.float32, tag="u", name="u", bufs=1)
                nc.vector.tensor_scalar(u[:], xf[:], 2.5, 5.5, op.mult, op.add)
                hats = []
                for i in range(1, 11):
                    z = tmp.tile([128, SLAB], dt.float32, tag="z", name="z", bufs=1)
                    nc.scalar.activation(z[:], xf[:], AF.Abs, bias=habias[i][:], scale=2.5)
                    hh = tmp.tile([128, SLAB], dt.float32, tag=f"h{i}", name=f"h{i}", bufs=1)
                    nc.scalar.activation(hh[:], z[:], AF.Relu, bias=1.0, scale=-1.0)
                    hats.append(hh)
                b2 = []
                for i in range(9):
                    ta = tmp.tile([128, SLAB], dt.float32, tag="ta", name="ta")
                    tb = tmp.tile([128, SLAB], dt.float32, tag="tb", name="tb")
                    nc.vector.scalar_tensor_tensor(ta[:], u[:], float(i), hats[i][:], op.subtract, op.mult)
                    nc.vector.scalar_tensor_tensor(tb[:], u[:], float(i + 3), hats[i + 1][:], op.subtract, op.mult)
                    bb = tmp.tile([128, SLAB], dt.float32, tag=f"b2_{i}", name=f"b2_{i}", bufs=1)
                    nc.gpsimd.tensor_tensor(bb[:], ta[:], tb[:], op.subtract)
                    b2.append(bb)
                for c in range(8):
                    ta = tmp.tile([128, SLAB], dt.float32, tag="ta", name="ta")
                    tb = tmp.tile([128, SLAB], dt.float32, tag="tb", name="tb")
                    nc.vector.scalar_tensor_tensor(ta[:], u[:], float(c), b2[c][:], op.subtract, op.mult)
                    nc.vector.scalar_tensor_tensor(tb[:], u[:], float(c + 4), b2[c + 1][:], op.subtract, op.mult)
                    nc.gpsimd.tensor_tensor(bs_sb[fb][c][:], ta[:], tb[:], op.subtract)

            def chunk_lhsT(t, tt):
                src = silu_sb[t] if t < 8 else bs_sb[(t - 8) // 8][(t - 8) % 8]
                return src[:, tt * 128:(tt + 1) * 128]

            # ---- group 0: [ke ko qe qo] matmul + k/q rope + a2a wave 1 ----
            pack_kq = [ropes.tile([128, 8 * 256], dt.bfloat16, tag=f"pkq{tt}", name=f"pkq{tt}")
                       for tt in range(2)]
            pack_v = [ropes.tile([128, 8 * 128], dt.bfloat16, tag=f"pv{tt}", name=f"pv{tt}")
                      for tt in range(2)]

            psA_cm = tc.tile_pool(name="psA", bufs=1, space=PSUM)
            psA = psA_cm.__enter__()
            qp = [[psA.tile([128, 512], dt.float32, tag=f"qkv{tt}_{ot}", name=f"qkv{tt}_{ot}")
                   for ot in range(4)] for tt in range(2)]
            for t in range(NCH):
                wt = wstream.tile([128, OKV], dt.bfloat16, tag="w0", name="w0", bufs=4)
                nc.sync.dma_start(wt[:], wm0_d[t])
                if t == 2 and rep == 0:
                    for tt in range(2):
                        nc.sync.dma_start(cs_sb[tt][:], cs_d[tt * 128:(tt + 1) * 128, :])
                        nc.sync.dma_start(sn_sb[tt][:], sn_d[tt * 128:(tt + 1) * 128, :])
                    nc.sync.dma_start(wo_sb[:], wo_d[:, :])
                for tt in range(2):
                    lhsT = chunk_lhsT(t, tt)
                    for ot in range(4):
                        nc.tensor.matmul(qp[tt][ot][:], lhsT, wt[:, ot * 512:(ot + 1) * 512],
                                         start=(t == 0), stop=(t == NCH - 1))

            def rope_pair(tt, ea, oa, base, pack_t, eng):
                """ea/oa: even/odd PSUM banks -> rotated into pack_t strided;
                real part lands at dest_blk+base+hp*64+[0:32], imag at +[32:64]."""
                blk = pack_t[:].rearrange("p (d q) -> p d q", d=8)[:, :, base:base + 128]
                blk = blk.rearrange("p d (hp i) -> p d hp i", hp=2)
                tg = "g" if eng is nc.gpsimd else ""
                t1 = tmp.tile([128, 512], dt.float32, tag=f"r1{tg}", name="r1")
                t2 = tmp.tile([128, 512], dt.float32, tag=f"r2{tg}", name="r2")
                eng.tensor_tensor(t1[:], ea[:], cs_sb[tt][:], op.mult)
                eng.tensor_tensor(t2[:], oa[:], sn_sb[tt][:], op.mult)
                eng.tensor_tensor(blk[:, :, :, 0:32], t1[:], t2[:], op.subtract)
                t3 = tmp.tile([128, 512], dt.float32, tag=f"r1{tg}", name="r1")
                t4 = tmp.tile([128, 512], dt.float32, tag=f"r2{tg}", name="r2")
                eng.tensor_tensor(t3[:], ea[:], sn_sb[tt][:], op.mult)
                eng.tensor_tensor(t4[:], oa[:], cs_sb[tt][:], op.mult)
                eng.tensor_tensor(blk[:, :, :, 32:64], t3[:], t4[:], op.add)

            # k ropes first: the v-group reuses the k banks (GPSIMD cannot
            # touch PSUM, so all rope products run on DVE)
            rope_pair(0, qp[0][0], qp[0][1], 0, pack_kq[0], nc.vector)
            rope_pair(1, qp[1][0], qp[1][1], 0, pack_kq[1], nc.vector)
            rope_pair(0, qp[0][2], qp[0][3], 128, pack_kq[0], nc.vector)
            rope_pair(1, qp[1][2], qp[1][3], 128, pack_kq[1], nc.vector)
            for tt in range(2):
                nc.sync.dma_start(
                    a2a_kq_i.ap()[:, tt * 128:(tt + 1) * 128, :].rearrange("d p q -> p d q"),
                    pack_kq[tt][:].rearrange("p (d q) -> p d q", d=8))
            if single_core:
                nc.gpsimd.dma_start(a2a_kq_o.ap(), a2a_kq_i.ap())
            else:
                nc.gpsimd.collective_compute(
                    "AllToAll", op.bypass, replica_groups=[list(range(CORES))],
                    ins=[a2a_kq_i.ap().opt()], outs=[a2a_kq_o.ap().opt()])

            # ---- group 1: [v] matmul; kq unpack + transposes interleaved ----
            qq = [[psA.tile([128, 512], dt.float32, tag=f"qkv{tt}_{ot}", name=f"qv{tt}_{ot}")
                   for ot in range(2)] for tt in range(2)]
            kqall = attn.tile([128, 16 * 256], dt.bfloat16, tag="kqall", name="kqall")
            ats_pre = []
            qT = attn.tile([128, S], dt.bfloat16, tag="qT", name="qT")
            kT = attn.tile([128, S], dt.bfloat16, tag="kT", name="kT")
            for t in range(NCH):
                wt = wstream.tile([128, OQ], dt.bfloat16, tag="w1", name="w1", bufs=4)
                nc.sync.dma_start(wt[:], wm1_d[t])
                if t == 4:
                    nc.sync.dma_start(
                        kqall[:].rearrange("p (s a q) -> p s a q", s=8, a=2),
                        a2a_kq_o.ap().rearrange("s (a p) q -> p s a q", a=2))
                for tt in range(2):
                    lhsT = chunk_lhsT(t, tt)
                    for ot in range(2):
                        nc.tensor.matmul(qq[tt][ot][:], lhsT, wt[:, ot * 512:(ot + 1) * 512],
                                         start=(t == 0), stop=(t == NCH - 1))
                if 32 <= t < 64:
                    idx = t - 32
                    st = idx % 16
                    is_q = idx >= 16
                    tp = psA.tile([128, 128], dt.bfloat16,
                                  tag=f"qkv{idx % 2}_{2 + (idx // 2) % 2}", name="tpk")
                    nc.tensor.matmul(tp[:], kqall[:, st * 256 + (128 if is_q else 0):
                                                  st * 256 + (256 if is_q else 128)],
                                     ident[:], is_transpose=True, skip_group_check=True)
                    nc.vector.tensor_copy((qT if is_q else kT)[:, st * 128:(st + 1) * 128], tp[:])
                elif t >= 64:
                    skc = t - 64
                    scp = psA.tile([128, 512], dt.float32,
                                   tag=f"qkv{skc % 2}_{2 + (skc // 2) % 2}", name="scp")
                    nc.tensor.matmul(scp[:], kT[0:64, skc * 128:(skc + 1) * 128],
                                     qT[0:64, 0:512], start=True, stop=True,
                                     skip_group_check=True)
                    ap = attnbuf.tile([128, 512], dt.bfloat16, tag=f"pre{skc}",
                                      name=f"pre{skc}", bufs=1)
                    nc.scalar.activation(ap[:], scp[:], AF.Exp)
                    ats_pre.append(ap)

            # v pack (no rope) + a2a wave 2 + vstat
            for tt in range(2):
                for b in (0, 1):
                    w = pack_v[tt][:].rearrange("p (d q) -> p d q", d=8)
                    w = w[:, b * 4:(b + 1) * 4, :]
                    dst = w.rearrange("p d (hp j) -> p d hp j", hp=2)
                    nc.vector.tensor_copy(dst, qq[tt][b][:])
            psA_cm.__exit__(None, None, None)
            for tt in range(2):
                nc.sync.dma_start(
                    a2a_v_i.ap()[:, tt * 128:(tt + 1) * 128, :].rearrange("d p q -> p d q"),
                    pack_v[tt][:].rearrange("p (d q) -> p d q", d=8))
            if single_core:
                nc.gpsimd.dma_start(a2a_v_o.ap(), a2a_v_i.ap())
            else:
                nc.gpsimd.collective_compute(
                    "AllToAll", op.bypass, replica_groups=[list(range(CORES))],
                    ins=[a2a_v_i.ap().opt()], outs=[a2a_v_o.ap().opt()])
            vstat = []
            for hp in range(2):
                vs = attn.tile([128, 16 * 65], dt.bfloat16, tag=f"vst{hp}", name=f"vst{hp}")
                v4 = vs[:].rearrange("p (s a j) -> p s a j", s=8, a=2)
                nc.sync.dma_start(
                    v4[:, :, :, 0:64],
                    a2a_v_o.ap().rearrange("s (a p) q -> p s a q", a=2)
                    [:, :, :, hp * 64:hp * 64 + 64])
                nc.vector.memset(v4[:, :, :, 64:65], 1.0)
                vstat.append(vs)

            # ---- attention + output projection ----
            psB_cm = tc.tile_pool(name="psB", bufs=1, space=PSUM)
            psB = psB_cm.__enter__()
            for sq in range(4):
                sqs = slice(sq * 512, (sq + 1) * 512)
                ctx_sb = attnbuf.tile([128, 512], dt.bfloat16, tag="ctx_sb", name="ctx_sb")
                ats = {}
                for hp in range(2):
                    hsl = slice(hp * 64, hp * 64 + 64)
                    # paired sk-chunks: one 2-bank PSUM tile, one exp for both
                    ats[hp] = [attnbuf.tile([128, 1024], dt.bfloat16, tag=f"at{hp}_{sm}",
                                            name=f"at{hp}_{sm}", bufs=1) for sm in range(8)]
                    for sm in range(8):
                        if sq == 0 and hp == 0 and sm < 4:
                            continue  # done as singles inside the v-group loop
                        sc = psB.tile([128, 1024], dt.float32, tag="sc", name="sc", bufs=2)
                        for half in range(2):
                            skc = 2 * sm + half
                            nc.tensor.matmul(sc[:, half * 512:(half + 1) * 512],
                                             kT[hsl, skc * 128:(skc + 1) * 128],
                                             qT[hsl, sqs], start=True, stop=True)
                        nc.scalar.activation(ats[hp][sm][:], sc[:], AF.Exp)
                for hp in range(2):
                    hsl = slice(hp * 64, hp * 64 + 64)
                    cx = psB.tile([65, 512], dt.float32, tag="cx", name="cx", bufs=2)
                    for skc in range(16):
                        if sq == 0 and hp == 0 and skc < 8:
                            src_ap = ats_pre[skc][:]
                        else:
                            src_ap = ats[hp][skc // 2][:, (skc % 2) * 512:(skc % 2 + 1) * 512]
                        nc.tensor.matmul(cx[:], vstat[hp][:, skc * 65:(skc + 1) * 65],
                                         src_ap, start=(skc == 0), stop=(skc == 15))
                    rcp = attnbuf.tile([1, 512], dt.float32, tag="rcp", name="rcp")
                    nc.vector.reciprocal(rcp[:], cx[64:65, :])
                    rb = attnbuf.tile([64, 512], dt.float32, tag="rb", name="rb")
                    nc.gpsimd.partition_broadcast(rb[:], rcp[:])
                    nc.vector.tensor_tensor(ctx_sb[hsl, :], cx[0:64, :], rb[:], op.mult)
                for tk in range(4):
                    for oh in range(2):
                        pr = psB.tile([128, 512], dt.float32, tag="pr", name="pr", bufs=1)
                        nc.tensor.matmul(pr[:], ctx_sb[:, tk * 128:(tk + 1) * 128],
                                         wo_sb[:, oh * 512:(oh + 1) * 512], start=True, stop=True)
                        po = attnbuf.tile([128, 512], dt.float32, tag="po", name="po")
                        nc.vector.tensor_copy(po[:], pr[:])
                        nc.sync.dma_start(out_d[sq * 512 + tk * 128: sq * 512 + (tk + 1) * 128,
                                                oh * 512:(oh + 1) * 512], po[:])
            psB_cm.__exit__(None, None, None)

    nc.compile()
    return nc


_NC = None


def _get_program():
    global _NC
    if _NC is None:
        _NC = _build_program()
    return _NC


def kernel(**inputs):
    x = inputs["x"]
    out_b = np.asarray(inputs["out_b"], np.float32)
    in_maps = _host_prep(x, inputs["base_weight"], inputs["spline_weight"],
                         inputs["spline_scaler"], inputs["out_w"],
                         inputs["rot_cos"], inputs["rot_sin"])
    nc = _get_program()

    if os.environ.get("KAN_SIM"):
        results = _run_sim(nc, in_maps)
    else:
        from concourse.bass_utils import run_bass_kernel_spmd
        res = run_bass_kernel_spmd(nc, in_maps, core_ids=list(range(CORES)))
        kernel.last_results = res
        results = res.results

    out = np.zeros((S, F), np.float64)
    for c in range(CORES):
        out += np.asarray(results[c]["out"], np.float32)
    out = out.astype(np.float32) + out_b[None, :]
    return out.reshape(1, S, F)


def _run_sim(nc, in_maps):
    from concourse.bass_interp import MultiCoreSim
    sim = MultiCoreSim(nc, num_cores=CORES, num_workers=CORES)
    for c in range(CORES):
        core = sim.cores[c]
        for k, v in in_maps[c].items():
            core.tensor(k)[:] = v
    sim.simulate()
    return [{"out": np.array(sim.cores[c].tensor("out"))} for c in range(CORES)]


def make_timed_runner(in_maps=None, nc=None):
    """Device-resident jitted runner (mirrors bass2jax.run_bass_via_pjrt,
    no output donation) for repeat-timing the NEFF execution."""
    import time
    import jax
    import concourse.mybir as mybir
    from jax.sharding import Mesh, PartitionSpec, NamedSharding
    from jax.experimental.shard_map import shard_map
    from concourse import bass2jax

    nc = nc or _get_program()
    bass2jax.install_neuronx_cc_hook()
    partition_name = nc.partition_id_tensor.name if nc.partition_id_tensor else None
    in_names, out_names, out_avals, zero_outs = [], [], [], []
    for alloc in nc.m.functions[0].allocations:
        if not isinstance(alloc, mybir.MemoryLocationSet):
            continue
        name = alloc.memorylocations[0].name
        if alloc.kind == "ExternalInput":
            if name != partition_name:
                in_names.append(name)
        elif alloc.kind == "ExternalOutput":
            shape = tuple(alloc.tensor_shape)
            dtype = mybir.dt.np(alloc.dtype)
            out_names.append(name)
            out_avals.append(jax.core.ShapedArray(shape, dtype))
            zero_outs.append(np.zeros(shape, dtype))
    n_params = len(in_names)
    all_in = in_names + out_names
    if partition_name is not None:
        all_in.append(partition_name)

    def _body(*args):
        operands = list(args)
        if partition_name is not None:
            operands.append(bass2jax.partition_id_tensor())
        return tuple(bass2jax._bass_exec_p.bind(
            *operands, out_avals=tuple(out_avals), in_names=tuple(all_in),
            out_names=tuple(out_names), lowering_input_output_aliases=(),
            sim_require_finite=True, sim_require_nnan=True, nc=nc))

    devices = jax.devices()[:CORES]
    mesh = Mesh(np.asarray(devices), ("core",))
    nsh = NamedSharding(mesh, PartitionSpec("core"))
    sharded = jax.jit(shard_map(_body, mesh=mesh,
                                in_specs=(PartitionSpec("core"),) * (n_params + len(out_names)),
                                out_specs=(PartitionSpec("core"),) * len(out_names),
                                check_rep=False), keep_unused=True)
    concat_in = [np.concatenate([np.asarray(in_maps[c][k]) for c in range(CORES)], axis=0)
                 for k in in_names]
    concat_zero = [np.zeros((CORES * z.shape[0], *z.shape[1:]), z.dtype) for z in zero_outs]
    dev_args = [jax.device_put(a, nsh) for a in concat_in + concat_zero]

    def run_once():
        t0 = time.perf_counter()
        outs = sharded(*dev_args)
        jax.block_until_ready(outs)
        return time.perf_counter() - t0, outs

    return run_once, out_names, out_avals

